# revision 23
# baseline (speedup 1.0000x reference)
# Bidirectional LSTM (T=128, B=128, NIN=NH=NOUT=512) on 8 trn2 NeuronCores.
#
# Sharding: 8 cores = 2 directions x 4 batch-quarters (B_loc=32). Fully
# symmetric SPMD program (no collectives): reverse-direction cores receive
# time-flipped inputs; the host flips their outputs back and sums the two
# directions' output-projection partials (+ b_emb).
#
# Per-core program:
#   phase 1: xp[t] = W_ih @ x_t + (b_ih + b_hh)  for all t  (bf16 matmuls,
#            f32 accum, spilled to an internal DRAM buffer)
#   phase 2: 128-step recurrence. Gates for step t are accumulated in PSUM:
#            an identity-weight matmul seeds xp_t, then 64 small matmuls add
#            W_hh @ h_{t-1}. Gates are split into two half-banks so ACT/DVE
#            elementwise for one half overlaps PE matmuls for the other.
#   phase 3: out_partial = W_emb_dir @ h  from the SBUF-resident h history.
import os
import sys

for _p in ("/opt/trn_rl_repo", "/root/.axon_site/_ro/trn_rl_repo"):
    if os.path.isdir(_p) and _p not in sys.path:
        sys.path.insert(0, _p)

import ml_dtypes
import numpy as np

import concourse.bass as bass  # noqa: F401  (registers bass types)
import concourse.mybir as mybir
import concourse.tile as tile
from concourse import bacc
from concourse.bass_utils import run_bass_kernel_spmd

BF16_NP = ml_dtypes.bfloat16
F32 = mybir.dt.float32
BF16 = mybir.dt.bfloat16
AF = mybir.ActivationFunctionType

T = 128
B = 128
NIN = 512
NH = 512
G = 4 * NH  # 2048 gate rows
NOUT = 512
NCORES = 8
BL = B // 4  # 32 batch rows per core (4 quarters x 2 directions)
TOK = T * BL  # 4096 tokens per core
NB = TOK // 512  # 8 token blocks of 512
KCH = NIN // 128  # 4 contraction chunks
MT = G // 128  # 16 gate M-tiles

# Gate-region permutation: PyTorch rows are (i, f, g, o); we lay out gate
# M-tiles in region order (i, f, o, g) so sigmoid covers one contiguous
# range and tanh another.
_PERM = np.r_[0:512, 512:1024, 1536:2048, 1024:1536]


def _build_program(floor=False, variant="full", loop_repeat=None, v2=False, v3=False):
    nc = bacc.Bacc("TRN2", target_bir_lowering=False, debug=False)
    xT_d = nc.dram_tensor("xT", [NIN, TOK], BF16, kind="ExternalInput").ap()
    wi_d = nc.dram_tensor("wiT", [NIN, G], BF16, kind="ExternalInput").ap()
    wh_d = nc.dram_tensor("whT", [NH, G], BF16, kind="ExternalInput").ap()
    we_d = nc.dram_tensor("weT", [NH, NOUT], BF16, kind="ExternalInput").ap()
    b_d = nc.dram_tensor("biasM", [128, MT], F32, kind="ExternalInput").ap()
    i_d = nc.dram_tensor("ident", [128, 128], BF16, kind="ExternalInput").ap()
    out_d = nc.dram_tensor("outT", [NOUT, TOK], F32, kind="ExternalOutput").ap()

    if floor:
        # Same external I/O, near-empty body: used by test.py to measure the
        # per-call dispatch floor so kernel HW time = full - floor.
        with tile.TileContext(nc) as tc:
            with tc.tile_pool(name="f", bufs=1) as fp:
                t_ = fp.tile([128, 512], BF16, tag="t")
                nc.sync.dma_start(out=t_, in_=xT_d[0:128, 0:512])
                t2_ = fp.tile([128, 512], F32, tag="t2")
                nc.vector.tensor_copy(t2_, t_)
                nc.sync.dma_start(out=out_d[0:128, 0:512], in_=t2_)
        nc.compile()
        return nc

    with tile.TileContext(nc) as tc:
        with tc.tile_pool(name="consts", bufs=1) as consts, \
             tc.tile_pool(name="xtk", bufs=2) as xtkp, \
             tc.tile_pool(name="xpst", bufs=4) as xpstp, \
             tc.tile_pool(name="gsb", bufs=3 if (v2 or v3) else 2) as gsbp, \
             tc.tile_pool(name="xpch", bufs=6 if (v2 or v3) else 3) as xpchp, \
             tc.tile_pool(name="small", bufs=3 if (v2 or v3) else 2) as smallp, \
             tc.tile_pool(name="osb", bufs=3) as osbp, \
             tc.tile_pool(name="psmm", bufs=2 if v2 else 4, space="PSUM") as psmm, \
             tc.tile_pool(name="psch", bufs=3 if v2 else 2, space="PSUM") as psch, \
             tc.tile_pool(name="xpdram", bufs=1, space="DRAM") as xpdr:

            # ---- constant loads ----
            wi_sb = consts.tile([128, KCH, G], BF16, tag="wi")
            nc.sync.dma_start(out=wi_sb, in_=wi_d.rearrange("(k p) m -> p k m", p=128))
            wh_sb = consts.tile([128, KCH, G], BF16, tag="wh")
            nc.sync.dma_start(out=wh_sb, in_=wh_d.rearrange("(k p) m -> p k m", p=128))
            we_sb = consts.tile([128, KCH, NOUT], BF16, tag="we")
            nc.sync.dma_start(out=we_sb, in_=we_d.rearrange("(j p) o -> p j o", p=128))
            b_sb = consts.tile([128, MT], F32, tag="bias")
            nc.sync.dma_start(out=b_sb, in_=b_d)
            id_sb = consts.tile([128, 128], BF16, tag="ident")
            nc.sync.dma_start(out=id_sb, in_=i_d)
            hh = consts.tile([128, T * 128], BF16, tag="hh")  # h history

            # xp spill, laid out so the chain's per-step gather is a 3-dim
            # slice: [half][partition][g*2+j'][token]  (m-tile m = g*4+2*half+j')
            xp_dram = xpdr.tile([2, 128, 8, TOK], BF16, tag="xp")

            # Optional hardware loop around the whole body (timing
            # amplification for benchmarking: one NEFF runs the body R times).
            import contextlib
            rep_cm = tc.For_i(0, loop_repeat, 1) if loop_repeat else contextlib.nullcontext()
            with rep_cm:
                _emit_body(nc, tc, variant, locals(), v2=v2)
    nc.compile()
    return nc


def _emit_body(nc, tc, variant, env, v2=False):
    xT_d = env["xT_d"]
    wi_sb = env["wi_sb"]; wh_sb = env["wh_sb"]; we_sb = env["we_sb"]
    b_sb = env["b_sb"]; id_sb = env["id_sb"]; hh = env["hh"]
    xp_dram = env["xp_dram"]; out_d = env["out_d"]
    xtkp = env["xtkp"]; xpstp = env["xpstp"]; gsbp = env["gsbp"]
    xpchp = env["xpchp"]; smallp = env["smallp"]; osbp = env["osbp"]
    psmm = env["psmm"]; psch = env["psch"]
    if True:
            # ---- phase 1: input projections ----
            xT_r = xT_d.rearrange("(k p) t -> p k t", p=128)
            for nb in range(NB):
                xt = xtkp.tile([128, KCH, 512], BF16, tag="xt")
                nc.sync.dma_start(out=xt, in_=xT_r[:, :, 512 * nb:512 * (nb + 1)])
                for m in range(MT):
                    psx = psmm.tile([128, 512], F32, tag="psmm")
                    for k in range(KCH):
                        nc.tensor.matmul(psx, wi_sb[:, k, 128 * m:128 * (m + 1)],
                                         xt[:, k, :], start=(k == 0), stop=(k == KCH - 1))
                    xpt = xpstp.tile([128, 512], BF16, tag="xpst")
                    if m % 2 == 0:
                        nc.scalar.activation(xpt, psx, AF.Identity, bias=b_sb[:, m:m + 1])
                    else:
                        nc.vector.tensor_scalar_add(xpt, psx, b_sb[:, m:m + 1])
                    g_, j_ = divmod(m, 4)
                    half_, j2_ = divmod(j_, 2)
                    nc.sync.dma_start(
                        out=xp_dram[half_, :, 2 * g_ + j2_, 512 * nb:512 * (nb + 1)],
                        in_=xpt)

            # ---- phase 2: recurrence ----
            c_prev = []
            for half in range(2):
                ct = smallp.tile([128, 64], F32, tag=f"c{half}")
                nc.vector.memset(ct, 0.0)
                c_prev.append(ct)
            if variant == "consth":
                nc.vector.memset(hh, 0.25)

            for t in range(T):
                xph = []
                for half in range(2):
                    xt_ = xpchp.tile([128, 8, 32], BF16, tag=f"xpch{half}")
                    nc.sync.dma_start(out=xt_,
                                      in_=xp_dram[half, :, :, 32 * t:32 * (t + 1)])
                    xph.append(xt_)
                ps = []
                for half in range(2):
                    pst = psch.tile([128, 256], F32, tag=f"psch{half}")
                    nc.tensor.matmul(pst, id_sb, xph[half],
                                     start=True, stop=(t == 0))
                    ps.append(pst)
                if t > 0 and variant != "nomm":
                    hprev = hh[:, 128 * (t - 1):128 * t]
                    for kp in range(2):  # k in {0,1} then {2,3}
                        for half in range(2):
                            for g in range(4):
                                for j2 in range(2):
                                    m = g * 4 + 2 * half + j2
                                    col = 64 * g + 32 * j2
                                    for k in (2 * kp, 2 * kp + 1):
                                        nc.tensor.matmul(
                                            ps[half][:, col:col + 32],
                                            wh_sb[:, k, 128 * m:128 * (m + 1)],
                                            hprev[:, 32 * k:32 * (k + 1)],
                                            start=False, stop=(k == KCH - 1))
                for half in range(2):
                    g_sb = gsbp.tile([128, 256], F32, tag=f"g{half}")
                    nc.scalar.activation(g_sb[:, 0:192], ps[half][:, 0:192], AF.Sigmoid)
                    nc.scalar.activation(g_sb[:, 192:256], ps[half][:, 192:256], AF.Tanh)
                    if variant == "consth":
                        if t == T - 1:
                            nc.sync.dma_start(
                                out=out_d[0:128, 256 * half:256 * (half + 1)],
                                in_=g_sb)
                        continue
                    t1 = smallp.tile([128, 64], F32, tag=f"t1{half}")
                    nc.vector.tensor_mul(t1, g_sb[:, 0:64], g_sb[:, 192:256])  # i*g
                    t2 = smallp.tile([128, 64], F32, tag=f"t2{half}")
                    if v2:  # f*c on the otherwise-idle GPSIMD engine
                        nc.gpsimd.tensor_mul(t2, g_sb[:, 64:128], c_prev[half])
                    else:
                        nc.vector.tensor_mul(t2, g_sb[:, 64:128], c_prev[half])
                    cn = smallp.tile([128, 64], F32, tag=f"c{half}")
                    nc.vector.tensor_add(cn, t1, t2)
                    th = smallp.tile([128, 64], F32, tag=f"th{half}")
                    nc.scalar.activation(th, cn, AF.Tanh)
                    nc.vector.tensor_mul(hh[:, 128 * t + 64 * half:128 * t + 64 * half + 64],
                                         g_sb[:, 128:192], th)                 # o*tanh(c)
                    c_prev[half] = cn

            # ---- phase 3: output projection ----
            hh_v = hh.rearrange("p (t j b) -> p t j b", j=KCH, b=32)
            for m in range(NOUT // 128):
                for nb in range(NB):
                    pso = psmm.tile([128, 512], F32, tag="psmm")
                    for j in range(KCH):
                        rhs = hh_v[:, 16 * nb:16 * (nb + 1), j, :]
                        nc.tensor.matmul(pso, we_sb[:, j, 128 * m:128 * (m + 1)],
                                         rhs, start=(j == 0), stop=(j == KCH - 1))
                    ot = osbp.tile([128, 512], F32, tag="osb")
                    if (m * NB + nb) % 2 == 0:
                        nc.scalar.copy(ot, pso)
                    else:
                        nc.vector.tensor_copy(ot, pso)
                    nc.sync.dma_start(
                        out=out_d[128 * m:128 * (m + 1), 512 * nb:512 * (nb + 1)], in_=ot)


_NC_CACHE = None


def _get_nc():
    global _NC_CACHE
    if _NC_CACHE is None:
        _NC_CACHE = _build_program()
    return _NC_CACHE


def make_in_maps(x, W_ih_f, W_hh_f, b_ih_f, b_hh_f,
                 W_ih_r, W_hh_r, b_ih_r, b_hh_r, W_emb, b_emb):
    """Host-side sharding/layout prep -> per-core input maps (8 cores)."""
    f32 = np.float32

    def dir_weights(W_ih, W_hh, b_ih, b_hh, we_cols):
        wiT = np.ascontiguousarray(W_ih.astype(f32)[_PERM].T).astype(BF16_NP)
        whT = np.ascontiguousarray(W_hh.astype(f32)[_PERM].T).astype(BF16_NP)
        bias = (b_ih.astype(f32) + b_hh.astype(f32))[_PERM]
        biasM = np.ascontiguousarray(bias.reshape(MT, 128).T)
        weT = np.ascontiguousarray(we_cols.astype(f32).T).astype(BF16_NP)
        return wiT, whT, biasM, weT

    wf = dir_weights(W_ih_f, W_hh_f, b_ih_f, b_hh_f, W_emb[:, :NH])
    wr = dir_weights(W_ih_r, W_hh_r, b_ih_r, b_hh_r, W_emb[:, NH:])
    ident = np.eye(128, dtype=BF16_NP)

    x_f32 = x.astype(f32)
    in_maps = []
    for core in range(NCORES):
        direction, q = divmod(core, 4)
        xs = x_f32[:, BL * q:BL * (q + 1), :]
        if direction == 1:
            xs = xs[::-1]
        # xT[feat, t*BL + b] = xs[t, b, feat]
        xT = np.ascontiguousarray(xs.transpose(2, 0, 1).reshape(NIN, TOK)).astype(BF16_NP)
        wiT, whT, biasM, weT = wf if direction == 0 else wr
        in_maps.append({"xT": xT, "wiT": wiT, "whT": whT, "weT": weT,
                        "biasM": biasM, "ident": ident})
    return in_maps


def assemble_output(results, b_emb):
    """Combine 8 per-core outT partials into the full (T, B, NOUT) output."""
    out = np.empty((T, B, NOUT), np.float32)
    for q in range(4):
        pf = results[q]["outT"].reshape(NOUT, T, BL).transpose(1, 2, 0)
        pr = results[4 + q]["outT"].reshape(NOUT, T, BL)[:, ::-1, :].transpose(1, 2, 0)
        out[:, BL * q:BL * (q + 1), :] = pf + pr
    out += b_emb.astype(np.float32)
    return out


def kernel(x, W_ih_f, W_hh_f, b_ih_f, b_hh_f,
           W_ih_r, W_hh_r, b_ih_r, b_hh_r, W_emb, b_emb):
    nc = _get_nc()
    in_maps = make_in_maps(x, W_ih_f, W_hh_f, b_ih_f, b_hh_f,
                           W_ih_r, W_hh_r, b_ih_r, b_hh_r, W_emb, b_emb)
    res = run_bass_kernel_spmd(nc, in_maps, list(range(NCORES)))
    return assemble_output(res.results, b_emb)
